# revision 20
# baseline (speedup 1.0000x reference)
"""Trainium2 Bass kernel for the box-smoothed Charbonnier loss.

reference:  diff = conv7x7_box(sum_ch(x - y)) / 49 ;  loss = mean(sqrt(diff^2 + 1e-6))

Strategy (pure data parallel, 2 images per core on 8 cores):
  - SWDGE cast-DMAs (f32 read -> bf16 SBUF write: all 3 channels of one
    128-row strip per DMA; the last strip is split into two w-halves so
    the tail chain overlaps the final transfer).  The f32->bf16 cast
    rides the DMA datapath, so the DVE diff chain runs at the 2x 16-bit
    rate and SBUF holds the inputs at half size.
  - The [128, 520] band (1/7 taps at |p - j + 4| <= 3, zeros elsewhere)
    is generated on gpsimd BEFORE the DMA issues: queued behind them it
    would not be ready until ~29us, stalling every stage-1 matmul;
    ahead of them it fits in the NEFF preamble gap at ~6-8us.
  - Software-pipelined schedule over global strips k = 0..7 with one
    batch of slack on every producer->consumer edge, so no engine queue
    ever waits mid-stream:
        batch k:  PE zero_ps2(k+2) | DVE diff k | PE stage1 k
                | ACT copies w(k-1) | PE stage2 w(k-2) | DVE reduce w(k-2)
    (stage-2 consumes copies made one batch earlier, reductions consume
    stage-2 done in the same batch but late on the DVE queue.)
  - stage 1 (H-conv) is a banded matmul per (strip, 128-col group g):
    stationary s[:, c, 128g:128g+128].  Each strip owns one psum bank
    T_k = [128, 4g, 128] covering output rows [128k, 128k+128); the
    +-3 row spill into neighbour strips' rows is two extra narrow
    matmuls accumulating into the neighbour banks (the into-next-strip
    spill deferred until that bank's start=True matmul ran).  T_k is
    final at batch k+1 (T_7 at batch 7).  NOTE (HW-verified):
    start=True resets the accumulate (has_written) state of the WHOLE
    psum bank, so only the bank's first matmul carries start=True; all
    later writes use start=False (fresh words -> plain write, armed
    words -> accumulate).
  - stage 2 (W-conv) per row window w: 16 matmuls of [128, 32]
    stationaries (from the ACT-copied SBUF mirror t) at psum partition
    offsets 32*hb into a per-window bank pre-zeroed two batches ahead
    (cheap matmul streaming the band's all-zero region), then one
    abs+sum reduction into the accumulator.  PSUM: 3 rotating T tags +
    5 rotating ps2 tags = 8 banks, with every rotation >= 1 batch clear
    of its previous tenant.
  - batch 7 runs in two half-strip chains (g-pairs) and drains windows
    5..7 with split DVE/ACT reductions; accumulator columns ship per
    image on the idle HWDGE rings.
  - Charbonnier: sqrt(d^2 + 1e-6) == |d| to ~1e-5 relative here.
  - The band is bf16(1/7) per stage; the host divides it back out and
    applies the exact 1/49.  The column bookkeeping is exact: stage-1
    stationaries are contiguous column blocks, so stage-2's contraction
    pairs column 128g+p with band(128g+p, n): true per-pixel conv.
"""

import numpy as np

import concourse.bass as bass
import concourse.bacc as bacc
import concourse.mybir as mybir
import concourse.tile as tile
from concourse.bass_interp import get_hw_module
from concourse.bass_utils import run_bass_kernel_spmd

N_CORES = 8
B_TOTAL = 16
B_PER_CORE = B_TOTAL // N_CORES
CH = 3
H = W = 512
P = 128
NC4 = 4
NSTRIP = B_PER_CORE * NC4  # 8 global strips / windows
EPS = 1e-6
F32 = mybir.dt.float32
BF16 = mybir.dt.bfloat16
# bf16 rounding of 1/7 (one factor per conv stage); host divides it back out
BAND_BF16 = 0.142578125
AF = mybir.ActivationFunctionType
BANDW = 520  # band free width: live window [0,136) + zeros through 520


def build_program():
    nc = bacc.Bacc("TRN2", target_bir_lowering=False, debug=False, num_devices=N_CORES)

    x = nc.dram_tensor("x", [B_PER_CORE, CH, H, W], F32, kind="ExternalInput")
    y = nc.dram_tensor("y", [B_PER_CORE, CH, H, W], F32, kind="ExternalInput")
    OUT_COLS = B_PER_CORE * 6
    out = nc.dram_tensor("out", [P, OUT_COLS], F32, kind="ExternalOutput")

    with tile.TileContext(nc) as tc:
        with (
            tc.tile_pool(name="const", bufs=1) as cpool,
            tc.tile_pool(name="xy", bufs=1) as xypool,
            tc.tile_pool(name="data", bufs=2) as dpool,
            tc.tile_pool(name="small", bufs=2) as spool,
            tc.tile_pool(name="psum", bufs=1, space="PSUM") as ppool,
        ):
            # ---- input DMAs: SWDGE cast-DMAs to bf16 ----
            xt, yt = [], []
            for b in range(B_PER_CORE):
                xb = xypool.tile([P, CH, NC4, W], BF16, name=f"xb{b}", tag=f"x{b}")
                yb = xypool.tile([P, CH, NC4, W], BF16, name=f"yb{b}", tag=f"y{b}")
                xt.append(xb)
                yt.append(yb)
            def issue_strip(b, c):
                src_x = x.ap()[b].rearrange("ch (c p) w -> p ch c w", c=NC4)
                src_y = y.ap()[b].rearrange("ch (c p) w -> p ch c w", c=NC4)
                if b == B_PER_CORE - 1 and c == NC4 - 1:
                    hw_ = W // 2
                    for h in range(2):
                        cs = slice(h * hw_, (h + 1) * hw_)
                        nc.gpsimd.dma_start(
                            xt[b][:, :, c, cs], src_x[:, :, c, cs])
                        nc.gpsimd.dma_start(
                            yt[b][:, :, c, cs], src_y[:, :, c, cs])
                else:
                    nc.gpsimd.dma_start(xt[b][:, :, c, :], src_x[:, :, c, :])
                    nc.gpsimd.dma_start(yt[b][:, :, c, :], src_y[:, :, c, :])

            issue_strip(0, 0)
            issue_strip(0, 1)
            # ---- band: after 2 strip-pairs' issues, before the rest ----
            sev = cpool.tile([P, 1], F32, name="sev")
            nc.gpsimd.memset(sev[:], BAND_BF16)
            band = cpool.tile([P, BANDW], BF16, name="band")
            btmp = cpool.tile([P, BANDW], BF16, name="btmp")
            ge = mybir.AluOpType.is_ge
            # keep where p - j + 7 >= 0
            nc.gpsimd.affine_select(
                btmp[:], sev[:].to_broadcast([P, BANDW]),
                pattern=[[-1, BANDW]], base=7, channel_multiplier=1,
                compare_op=ge, fill=0.0,
            )
            # keep where -p + j - 1 >= 0
            nc.gpsimd.affine_select(
                band[:], btmp[:],
                pattern=[[1, BANDW]], base=-1, channel_multiplier=-1,
                compare_op=ge, fill=0.0,
            )

            for b in range(B_PER_CORE):
                for c in range(NC4):
                    if not (b == 0 and c < 2):
                        issue_strip(b, c)

            acc_v = cpool.tile([P, 4], F32, name="accv")
            acc_s = cpool.tile([P, NSTRIP], F32, name="accs")
            col_v = 0
            col_s = 0
            out_col = 0

            prev = {}

            def ordered(key, inst):
                # pin each engine's queue to data-arrival order
                if key in prev:
                    tile.add_dep_helper(inst.ins, prev[key], sync=False,
                                        reason=f"{key} arrival order")
                prev[key] = inst.ins
                return inst

            # per-image s (diff) and t (H-conv mirror) tiles
            st = [dpool.tile([P, NC4, W], BF16, name=f"s{b}", tag="s")
                  for b in range(B_PER_CORE)]
            tt = [dpool.tile([P, NC4, W], BF16, name=f"t{b}", tag="t")
                  for b in range(B_PER_CORE)]
            Tg = [None] * NSTRIP   # stage-1 psum bank per global strip
            ps2 = [None] * NSTRIP  # stage-2 psum bank per global window

            def diff_strip(k, w0, w1):
                b, c = divmod(k, NC4)
                xb, yb, sv = xt[b], yt[b], st[b]
                ww = w1 - w0
                d0 = spool.tile([P, ww], BF16, name="d0", tag=f"d0_{ww}")
                d1 = spool.tile([P, ww], BF16, name="d1", tag=f"d1_{ww}")
                e = spool.tile([P, ww], BF16, name="e", tag=f"e_{ww}")
                ordered("v", nc.vector.tensor_sub(
                    d0[:], xb[:, 0, c, w0:w1], yb[:, 0, c, w0:w1]))
                ordered("v", nc.vector.tensor_sub(
                    d1[:], xb[:, 1, c, w0:w1], yb[:, 1, c, w0:w1]))
                ordered("v", nc.vector.tensor_add(e[:], d0[:], d1[:]))
                ordered("v", nc.vector.tensor_sub(
                    d1[:], xb[:, 2, c, w0:w1], yb[:, 2, c, w0:w1]))
                ordered("v", nc.vector.tensor_add(sv[:, c, w0:w1], e[:], d1[:]))

            def zero_ps2(w):
                ps2[w] = ppool.tile([P, W], F32, name=f"ps2_{w}",
                                    tag=f"ps2{w % 5}")
                # band[:, 136:264] is all zeros
                ordered("t", nc.tensor.matmul(
                    ps2[w][:], band[:, 136:264], band[:, 0:W],
                    start=True, stop=False,
                ))

            def stage1(k, g_lo, g_hi):
                b, c = divmod(k, NC4)
                s = st[b]
                if g_lo == 0:
                    Tg[k] = ppool.tile([P, NC4, P], F32, name=f"T{k}",
                                       tag=f"T{k % 3}")
                for g in range(g_lo, g_hi):
                    ordered("t", nc.tensor.matmul(
                        Tg[k][:, g, :],
                        s[:, c, 128 * g:128 * (g + 1)],
                        band[:, 4:132],
                        start=(g == 0),
                        stop=False,
                    ))
                if c > 0:
                    for g in range(g_lo, g_hi):
                        # strip c-1 rows 125..127 -> our rows 0..2
                        ordered("t", nc.tensor.matmul(
                            Tg[k][:, g, 0:3],
                            s[:, c - 1, 128 * g:128 * (g + 1)],
                            band[:, 132:135],
                            start=False,
                            stop=(c == NC4 - 1),
                        ))
                    for g in range(g_lo, g_hi):
                        # our rows 0..2 -> strip c-1 rows 125..127
                        ordered("t", nc.tensor.matmul(
                            Tg[k - 1][:, g, 125:128],
                            s[:, c, 128 * g:128 * (g + 1)],
                            band[:, 1:4],
                            start=False,
                            stop=True,
                        ))

            def copies(w, g_lo, g_hi, split=False):
                # T_w (final) -> t rows [128c, 128c+128); ACT mid-stream,
                # split DVE/ACT in the tail where the DVE is idle
                b, c = divmod(w, NC4)
                for g in range(g_lo, g_hi):
                    dst = tt[b][:, g, 128 * c:128 * (c + 1)]
                    if split and g % 2 == 0:
                        ordered("v", nc.vector.tensor_copy(dst, Tg[w][:, g, :]))
                    else:
                        ordered("s", nc.scalar.copy(dst, Tg[w][:, g, :]))

            def stage2(w, g_lo, g_hi):
                b, c = divmod(w, NC4)
                t = tt[b]
                for g in range(g_lo, g_hi):
                    n0, n1 = max(0, 128 * g - 4), min(W, 128 * g + 132)
                    j0 = n0 - 128 * g + 4
                    j1 = n1 - 128 * g + 4
                    for hb in range(NC4):
                        ordered("t", nc.tensor.matmul(
                            ps2[w][32 * hb:32 * hb + 32, n0:n1],
                            t[:, g, 128 * c + hb:128 * (c + 1):NC4],
                            band[:, j0:j1],
                            start=False,
                            stop=(hb == NC4 - 1 and g == NC4 - 1),
                            tile_position=(0, 32 * hb),
                        ))

            def reduce_window(w, split):
                # mid-stream reductions live on ACT so the DVE runs ONLY
                # diffs and paces the arrivals; tail windows split DVE/ACT
                nonlocal col_v, col_s
                if split:
                    ordered("v", nc.vector.tensor_reduce(
                        acc_v[:, col_v:col_v + 1], ps2[w][:, 0:W // 2],
                        axis=mybir.AxisListType.X, op=mybir.AluOpType.add,
                        apply_absolute_value=True))
                    col_v += 1
                    u = spool.tile([P, W // 2], BF16, name="u", tag="u")
                    ordered("s", nc.scalar.activation(
                        u[:], ps2[w][:, W // 2:], AF.Abs,
                        accum_out=acc_s[:, col_s:col_s + 1]))
                    col_s += 1
                else:
                    u = spool.tile([P, W], BF16, name="u2", tag="u2")
                    ordered("s", nc.scalar.activation(
                        u[:], ps2[w][:], AF.Abs,
                        accum_out=acc_s[:, col_s:col_s + 1]))
                    col_s += 1

            def ship_image(b, v_start, s_start):
                nonlocal out_col
                nv, ns = col_v - v_start, col_s - s_start
                if nv:
                    nc.sync.dma_start(
                        out.ap()[:, out_col:out_col + nv],
                        acc_v[:, v_start:col_v])
                    out_col += nv
                if ns:
                    nc.scalar.dma_start(
                        out.ap()[:, out_col:out_col + ns],
                        acc_s[:, s_start:col_s])
                    out_col += ns

            img_marks = [(0, 0)]
            # ---- steady-state batches k = 0..6 ----
            # PE emission order puts stage2 w(k-2) BEFORE stage1 k: its
            # inputs (copies from batch k-1) are already in SBUF, so the
            # PE works through it while the DVE is still diffing strip k.
            for k in range(NSTRIP - 1):
                if k + 2 < NSTRIP:
                    zero_ps2(k + 2)
                if k == 0:
                    zero_ps2(0)
                    zero_ps2(1)
                diff_strip(k, 0, W)
                if k >= 2:
                    stage2(k - 2, 0, NC4)
                stage1(k, 0, NC4)
                if k >= 1:
                    copies(k - 1, 0, NC4)
                if k >= 2:
                    reduce_window(k - 2, split=False)
                if k == NC4 + 1:
                    # img0's last window (w3) just reduced: ship img0
                    img_marks.append((col_v, col_s))
                    ship_image(0, 0, 0)

            # ---- tail batch k = 7: two half-strip chains ----
            # DVE runs both half-diffs back to back (second waits only on
            # its own DMA), then the drain copies/reductions; the PE
            # dovetails stage2 w5 into the gap between the half stage-1s.
            kL = NSTRIP - 1
            diff_strip(kL, 0, W // 2)            # DVE: half a
            diff_strip(kL, W // 2, W)            # DVE: half b
            stage1(kL, 0, 2)                     # PE (after diff 7a)
            stage2(kL - 2, 0, NC4)               # PE: w5 (copies from batch 6)
            stage1(kL, 2, NC4)                   # PE (after diff 7b)
            copies(kL - 1, 0, 2, split=True)     # w6 g01
            copies(kL, 0, 2, split=True)         # w7 g01
            copies(kL - 1, 2, NC4, split=True)   # w6 g23
            copies(kL, 2, NC4, split=True)       # w7 g23
            stage2(kL - 1, 0, NC4)
            stage2(kL, 0, NC4)
            reduce_window(kL - 2, split=True)    # w5
            reduce_window(kL - 1, split=True)    # w6
            reduce_window(kL, split=True)        # w7
            v0, s0 = img_marks[1]
            ship_image(1, v0, s0)

            n_out_cols = out_col

    nc.compile()
    nc.m = get_hw_module(nc.m)
    return nc, x.name, y.name, out.name, n_out_cols


_CACHE = {}


def _get_program():
    if "prog" not in _CACHE:
        _CACHE["prog"] = build_program()
    return _CACHE["prog"]


def run_sharded(x: np.ndarray, y: np.ndarray, trace: bool = False):
    """Run the SPMD kernel; returns (per-core sums list, BassKernelResults)."""
    nc, xname, yname, outname, n_cols = _get_program()
    x = np.ascontiguousarray(np.asarray(x, dtype=np.float32))
    y = np.ascontiguousarray(np.asarray(y, dtype=np.float32))
    in_maps = []
    for k in range(N_CORES):
        sl = slice(k * B_PER_CORE, (k + 1) * B_PER_CORE)
        in_maps.append({
            xname: x[sl],
            yname: y[sl],
        })
    res = run_bass_kernel_spmd(
        nc, in_maps, core_ids=list(range(N_CORES)), trace=trace
    )
    sums = [float(res.results[k][outname][:, :n_cols]
                  .astype(np.float64).sum())
            for k in range(N_CORES)]
    return sums, res


def kernel(x: np.ndarray, y: np.ndarray) -> np.ndarray:
    sums, _ = run_sharded(x, y)
    total = float(np.sum(np.asarray(sums, dtype=np.float64)))
    # the device band carries bf16(1/7) per conv stage; divide it back out
    # and apply the exact 1/49 here
    total *= (1.0 / 49.0) / (BAND_BF16 * BAND_BF16)
    return np.float32(total / (B_TOTAL * H * W))


# revision 21
# speedup vs baseline: 1.0098x; 1.0098x over previous
"""Trainium2 Bass kernel for the box-smoothed Charbonnier loss.

reference:  diff = conv7x7_box(sum_ch(x - y)) / 49 ;  loss = mean(sqrt(diff^2 + 1e-6))

Strategy (pure data parallel, 2 images per core on 8 cores):
  - SWDGE cast-DMAs (f32 read -> bf16 SBUF write: all 3 channels of one
    128-row strip per DMA; the last strip is split into two w-halves so
    the tail chain overlaps the final transfer).  The f32->bf16 cast
    rides the DMA datapath, so the DVE diff chain runs at the 2x 16-bit
    rate and SBUF holds the inputs at half size.
  - The [128, 520] band (1/7 taps at |p - j + 4| <= 3, zeros elsewhere)
    is generated on gpsimd BEFORE the DMA issues: queued behind them it
    would not be ready until ~29us, stalling every stage-1 matmul;
    ahead of them it fits in the NEFF preamble gap at ~6-8us.
  - Software-pipelined schedule over global strips k = 0..7 with one
    batch of slack on every producer->consumer edge, so no engine queue
    ever waits mid-stream:
        batch k:  PE zero_ps2(k+2) | DVE diff k | PE stage1 k
                | ACT copies w(k-1) | PE stage2 w(k-2) | DVE reduce w(k-2)
    (stage-2 consumes copies made one batch earlier, reductions consume
    stage-2 done in the same batch but late on the DVE queue.)
  - stage 1 (H-conv) is a banded matmul per (strip, 128-col group g):
    stationary s[:, c, 128g:128g+128].  Each strip owns one psum bank
    T_k = [128, 4g, 128] covering output rows [128k, 128k+128); the
    +-3 row spill into neighbour strips' rows is two extra narrow
    matmuls accumulating into the neighbour banks (the into-next-strip
    spill deferred until that bank's start=True matmul ran).  T_k is
    final at batch k+1 (T_7 at batch 7).  NOTE (HW-verified):
    start=True resets the accumulate (has_written) state of the WHOLE
    psum bank, so only the bank's first matmul carries start=True; all
    later writes use start=False (fresh words -> plain write, armed
    words -> accumulate).
  - stage 2 (W-conv) per row window w: 16 matmuls of [128, 32]
    stationaries (from the ACT-copied SBUF mirror t) at psum partition
    offsets 32*hb into a per-window bank pre-zeroed two batches ahead
    (cheap matmul streaming the band's all-zero region), then one
    abs+sum reduction into the accumulator.  PSUM: 3 rotating T tags +
    5 rotating ps2 tags = 8 banks, with every rotation >= 1 batch clear
    of its previous tenant.
  - batch 7 runs in two half-strip chains (g-pairs) and drains windows
    5..7 with split DVE/ACT reductions; accumulator columns ship per
    image on the idle HWDGE rings.
  - Charbonnier: sqrt(d^2 + 1e-6) == |d| to ~1e-5 relative here.
  - The band is bf16(1/7) per stage; the host divides it back out and
    applies the exact 1/49.  The column bookkeeping is exact: stage-1
    stationaries are contiguous column blocks, so stage-2's contraction
    pairs column 128g+p with band(128g+p, n): true per-pixel conv.
"""

import numpy as np

import concourse.bass as bass
import concourse.bacc as bacc
import concourse.mybir as mybir
import concourse.tile as tile
from concourse.bass_interp import get_hw_module
from concourse.bass_utils import run_bass_kernel_spmd

N_CORES = 8
B_TOTAL = 16
B_PER_CORE = B_TOTAL // N_CORES
CH = 3
H = W = 512
P = 128
NC4 = 4
NSTRIP = B_PER_CORE * NC4  # 8 global strips / windows
EPS = 1e-6
F32 = mybir.dt.float32
BF16 = mybir.dt.bfloat16
# bf16 rounding of 1/7 (one factor per conv stage); host divides it back out
BAND_BF16 = 0.142578125
AF = mybir.ActivationFunctionType
BANDW = 520  # band free width: live window [0,136) + zeros through 520


def build_program():
    nc = bacc.Bacc("TRN2", target_bir_lowering=False, debug=False, num_devices=N_CORES)

    x = nc.dram_tensor("x", [B_PER_CORE, CH, H, W], F32, kind="ExternalInput")
    y = nc.dram_tensor("y", [B_PER_CORE, CH, H, W], F32, kind="ExternalInput")
    OUT_COLS = B_PER_CORE * 6
    out = nc.dram_tensor("out", [P, OUT_COLS], F32, kind="ExternalOutput")

    with tile.TileContext(nc) as tc:
        with (
            tc.tile_pool(name="const", bufs=1) as cpool,
            tc.tile_pool(name="xy", bufs=1) as xypool,
            tc.tile_pool(name="data", bufs=2) as dpool,
            tc.tile_pool(name="small", bufs=2) as spool,
            tc.tile_pool(name="psum", bufs=1, space="PSUM") as ppool,
        ):
            # ---- input DMAs: SWDGE cast-DMAs to bf16 ----
            xt, yt = [], []
            for b in range(B_PER_CORE):
                xb = xypool.tile([P, CH, NC4, W], BF16, name=f"xb{b}", tag=f"x{b}")
                yb = xypool.tile([P, CH, NC4, W], BF16, name=f"yb{b}", tag=f"y{b}")
                xt.append(xb)
                yt.append(yb)
            def issue_strip(b, c):
                src_x = x.ap()[b].rearrange("ch (c p) w -> p ch c w", c=NC4)
                src_y = y.ap()[b].rearrange("ch (c p) w -> p ch c w", c=NC4)
                if b == B_PER_CORE - 1 and c == NC4 - 1:
                    hw_ = W // 2
                    for h in range(2):
                        cs = slice(h * hw_, (h + 1) * hw_)
                        nc.gpsimd.dma_start(
                            xt[b][:, :, c, cs], src_x[:, :, c, cs])
                        nc.gpsimd.dma_start(
                            yt[b][:, :, c, cs], src_y[:, :, c, cs])
                else:
                    nc.gpsimd.dma_start(xt[b][:, :, c, :], src_x[:, :, c, :])
                    nc.gpsimd.dma_start(yt[b][:, :, c, :], src_y[:, :, c, :])

            issue_strip(0, 0)
            issue_strip(0, 1)
            # ---- band: after 2 strip-pairs' issues, before the rest ----
            sev = cpool.tile([P, 1], F32, name="sev")
            nc.gpsimd.memset(sev[:], BAND_BF16)
            band = cpool.tile([P, BANDW], BF16, name="band")
            btmp = cpool.tile([P, BANDW], BF16, name="btmp")
            ge = mybir.AluOpType.is_ge
            # keep where p - j + 7 >= 0
            nc.gpsimd.affine_select(
                btmp[:], sev[:].to_broadcast([P, BANDW]),
                pattern=[[-1, BANDW]], base=7, channel_multiplier=1,
                compare_op=ge, fill=0.0,
            )
            # keep where -p + j - 1 >= 0
            nc.gpsimd.affine_select(
                band[:], btmp[:],
                pattern=[[1, BANDW]], base=-1, channel_multiplier=-1,
                compare_op=ge, fill=0.0,
            )

            for b in range(B_PER_CORE):
                for c in range(NC4):
                    if not (b == 0 and c < 2):
                        issue_strip(b, c)

            acc = cpool.tile([P, 12], F32, name="acc")
            col = 0
            out_col = 0

            prev = {}

            def ordered(key, inst):
                # pin each engine's queue to data-arrival order
                if key in prev:
                    tile.add_dep_helper(inst.ins, prev[key], sync=False,
                                        reason=f"{key} arrival order")
                prev[key] = inst.ins
                return inst

            # per-image s (diff) and t (H-conv mirror) tiles
            st = [dpool.tile([P, NC4, W], BF16, name=f"s{b}", tag="s")
                  for b in range(B_PER_CORE)]
            tt = [dpool.tile([P, NC4, W], BF16, name=f"t{b}", tag="t")
                  for b in range(B_PER_CORE)]
            Tg = [None] * NSTRIP   # stage-1 psum bank per global strip
            ps2 = [None] * NSTRIP  # stage-2 psum bank per global window

            def diff_strip(k, w0, w1):
                b, c = divmod(k, NC4)
                xb, yb, sv = xt[b], yt[b], st[b]
                ww = w1 - w0
                d0 = spool.tile([P, ww], BF16, name="d0", tag=f"d0_{ww}")
                d1 = spool.tile([P, ww], BF16, name="d1", tag=f"d1_{ww}")
                e = spool.tile([P, ww], BF16, name="e", tag=f"e_{ww}")
                ordered("v", nc.vector.tensor_sub(
                    d0[:], xb[:, 0, c, w0:w1], yb[:, 0, c, w0:w1]))
                ordered("v", nc.vector.tensor_sub(
                    d1[:], xb[:, 1, c, w0:w1], yb[:, 1, c, w0:w1]))
                ordered("v", nc.vector.tensor_add(e[:], d0[:], d1[:]))
                ordered("v", nc.vector.tensor_sub(
                    d1[:], xb[:, 2, c, w0:w1], yb[:, 2, c, w0:w1]))
                ordered("v", nc.vector.tensor_add(sv[:, c, w0:w1], e[:], d1[:]))

            def zero_ps2(w):
                ps2[w] = ppool.tile([P, W], F32, name=f"ps2_{w}",
                                    tag=f"ps2{w % 5}")
                # band[:, 136:264] is all zeros
                ordered("t", nc.tensor.matmul(
                    ps2[w][:], band[:, 136:264], band[:, 0:W],
                    start=True, stop=False,
                ))

            def stage1(k, g_lo, g_hi):
                b, c = divmod(k, NC4)
                s = st[b]
                if g_lo == 0:
                    Tg[k] = ppool.tile([P, NC4, P], F32, name=f"T{k}",
                                       tag=f"T{k % 3}")
                for g in range(g_lo, g_hi):
                    ordered("t", nc.tensor.matmul(
                        Tg[k][:, g, :],
                        s[:, c, 128 * g:128 * (g + 1)],
                        band[:, 4:132],
                        start=(g == 0),
                        stop=False,
                    ))
                if c > 0:
                    for g in range(g_lo, g_hi):
                        # strip c-1 rows 125..127 -> our rows 0..2
                        ordered("t", nc.tensor.matmul(
                            Tg[k][:, g, 0:3],
                            s[:, c - 1, 128 * g:128 * (g + 1)],
                            band[:, 132:135],
                            start=False,
                            stop=(c == NC4 - 1),
                        ))
                    for g in range(g_lo, g_hi):
                        # our rows 0..2 -> strip c-1 rows 125..127
                        ordered("t", nc.tensor.matmul(
                            Tg[k - 1][:, g, 125:128],
                            s[:, c, 128 * g:128 * (g + 1)],
                            band[:, 1:4],
                            start=False,
                            stop=True,
                        ))

            def copies(w, g_lo, g_hi, split=False):
                # T_w (final) -> t rows [128c, 128c+128); ACT mid-stream,
                # split DVE/ACT in the tail where the DVE is idle
                b, c = divmod(w, NC4)
                for g in range(g_lo, g_hi):
                    dst = tt[b][:, g, 128 * c:128 * (c + 1)]
                    if split and g % 2 == 0:
                        ordered("v", nc.vector.tensor_copy(dst, Tg[w][:, g, :]))
                    else:
                        ordered("s", nc.scalar.copy(dst, Tg[w][:, g, :]))

            def stage2(w, g_lo, g_hi):
                b, c = divmod(w, NC4)
                t = tt[b]
                for g in range(g_lo, g_hi):
                    n0, n1 = max(0, 128 * g - 4), min(W, 128 * g + 132)
                    j0 = n0 - 128 * g + 4
                    j1 = n1 - 128 * g + 4
                    for hb in range(NC4):
                        ordered("t", nc.tensor.matmul(
                            ps2[w][32 * hb:32 * hb + 32, n0:n1],
                            t[:, g, 128 * c + hb:128 * (c + 1):NC4],
                            band[:, j0:j1],
                            start=False,
                            stop=(hb == NC4 - 1 and g == NC4 - 1),
                            tile_position=(0, 32 * hb),
                        ))

            def reduce_window(w, split):
                # mid-stream reductions live on ACT so the DVE runs ONLY
                # diffs and paces the arrivals; tail windows split DVE/ACT.
                # Both engines write disjoint columns of ONE acc tile so a
                # single sync-ring DMA ships everything.
                nonlocal col
                if split:
                    ordered("v", nc.vector.tensor_reduce(
                        acc[:, col:col + 1], ps2[w][:, 0:W // 2],
                        axis=mybir.AxisListType.X, op=mybir.AluOpType.add,
                        apply_absolute_value=True))
                    col += 1
                    u = spool.tile([P, W // 2], BF16, name="u", tag="u")
                    ordered("s", nc.scalar.activation(
                        u[:], ps2[w][:, W // 2:], AF.Abs,
                        accum_out=acc[:, col:col + 1]))
                    col += 1
                else:
                    u = spool.tile([P, W], BF16, name="u2", tag="u2")
                    ordered("s", nc.scalar.activation(
                        u[:], ps2[w][:], AF.Abs,
                        accum_out=acc[:, col:col + 1]))
                    col += 1

            def ship_image(b, c_start):
                nonlocal out_col
                n = col - c_start
                if n:
                    nc.sync.dma_start(
                        out.ap()[:, out_col:out_col + n],
                        acc[:, c_start:col])
                    out_col += n

            img_marks = [0]
            # ---- steady-state batches k = 0..6 ----
            # PE emission order puts stage2 w(k-2) BEFORE stage1 k: its
            # inputs (copies from batch k-1) are already in SBUF, so the
            # PE works through it while the DVE is still diffing strip k.
            for k in range(NSTRIP - 1):
                if k + 2 < NSTRIP:
                    zero_ps2(k + 2)
                if k == 0:
                    zero_ps2(0)
                    zero_ps2(1)
                diff_strip(k, 0, W)
                if k >= 2:
                    stage2(k - 2, 0, NC4)
                stage1(k, 0, NC4)
                if k >= 1:
                    copies(k - 1, 0, NC4)
                if k >= 2:
                    reduce_window(k - 2, split=False)
                if k == NC4 + 1:
                    # img0's last window (w3) just reduced: ship img0
                    img_marks.append(col)
                    ship_image(0, 0)

            # ---- tail batch k = 7: two half-strip chains ----
            # DVE runs both half-diffs back to back (second waits only on
            # its own DMA), then the drain copies/reductions; the PE
            # dovetails stage2 w5 into the gap between the half stage-1s.
            kL = NSTRIP - 1
            diff_strip(kL, 0, W // 2)            # DVE: half a
            diff_strip(kL, W // 2, W)            # DVE: half b
            stage1(kL, 0, 2)                     # PE (after diff 7a)
            stage2(kL - 2, 0, NC4)               # PE: w5 (copies from batch 6)
            stage1(kL, 2, NC4)                   # PE (after diff 7b)
            copies(kL - 1, 0, 2, split=True)     # w6 g01
            copies(kL, 0, 2, split=True)         # w7 g01
            copies(kL - 1, 2, NC4, split=True)   # w6 g23
            copies(kL, 2, NC4, split=True)       # w7 g23
            stage2(kL - 1, 0, NC4)
            stage2(kL, 0, NC4)
            reduce_window(kL - 2, split=True)    # w5
            reduce_window(kL - 1, split=True)    # w6
            reduce_window(kL, split=True)        # w7
            ship_image(1, img_marks[1])

            n_out_cols = out_col

    nc.compile()
    nc.m = get_hw_module(nc.m)
    return nc, x.name, y.name, out.name, n_out_cols


_CACHE = {}


def _get_program():
    if "prog" not in _CACHE:
        _CACHE["prog"] = build_program()
    return _CACHE["prog"]


def run_sharded(x: np.ndarray, y: np.ndarray, trace: bool = False):
    """Run the SPMD kernel; returns (per-core sums list, BassKernelResults)."""
    nc, xname, yname, outname, n_cols = _get_program()
    x = np.ascontiguousarray(np.asarray(x, dtype=np.float32))
    y = np.ascontiguousarray(np.asarray(y, dtype=np.float32))
    in_maps = []
    for k in range(N_CORES):
        sl = slice(k * B_PER_CORE, (k + 1) * B_PER_CORE)
        in_maps.append({
            xname: x[sl],
            yname: y[sl],
        })
    res = run_bass_kernel_spmd(
        nc, in_maps, core_ids=list(range(N_CORES)), trace=trace
    )
    sums = [float(res.results[k][outname][:, :n_cols]
                  .astype(np.float64).sum())
            for k in range(N_CORES)]
    return sums, res


def kernel(x: np.ndarray, y: np.ndarray) -> np.ndarray:
    sums, _ = run_sharded(x, y)
    total = float(np.sum(np.asarray(sums, dtype=np.float64)))
    # the device band carries bf16(1/7) per conv stage; divide it back out
    # and apply the exact 1/49 here
    total *= (1.0 / 49.0) / (BAND_BF16 * BAND_BF16)
    return np.float32(total / (B_TOTAL * H * W))


# revision 22
# speedup vs baseline: 1.0943x; 1.0837x over previous
"""Trainium2 Bass kernel for the box-smoothed Charbonnier loss.

reference:  diff = conv7x7_box(sum_ch(x - y)) / 49 ;  loss = mean(sqrt(diff^2 + 1e-6))

Strategy (pure data parallel, 2 images per core on 8 cores):
  - SWDGE cast-DMAs (f32 read -> bf16 SBUF write: all 3 channels of one
    128-row strip per DMA; the last strip is split into two w-halves so
    the tail chain overlaps the final transfer).  The f32->bf16 cast
    rides the DMA datapath, so the DVE diff chain runs at the 2x 16-bit
    rate and SBUF holds the inputs at half size.
  - The [128, 520] band (1/7 taps at |p - j + 4| <= 3, zeros elsewhere)
    is generated on gpsimd BEFORE the DMA issues: queued behind them it
    would not be ready until ~29us, stalling every stage-1 matmul;
    ahead of them it fits in the NEFF preamble gap at ~6-8us.
  - Software-pipelined schedule over global strips k = 0..7 with one
    batch of slack on every producer->consumer edge, so no engine queue
    ever waits mid-stream:
        batch k:  PE zero_ps2(k+2) | DVE diff k | PE stage1 k
                | ACT copies w(k-1) | PE stage2 w(k-2) | DVE reduce w(k-2)
    (stage-2 consumes copies made one batch earlier, reductions consume
    stage-2 done in the same batch but late on the DVE queue.)
  - stage 1 (H-conv) is a banded matmul per (strip, 128-col group g):
    stationary s[:, c, 128g:128g+128].  Each strip owns one psum bank
    T_k = [128, 4g, 128] covering output rows [128k, 128k+128); the
    +-3 row spill into neighbour strips' rows is two extra narrow
    matmuls accumulating into the neighbour banks (the into-next-strip
    spill deferred until that bank's start=True matmul ran).  T_k is
    final at batch k+1 (T_7 at batch 7).  NOTE (HW-verified):
    start=True resets the accumulate (has_written) state of the WHOLE
    psum bank, so only the bank's first matmul carries start=True; all
    later writes use start=False (fresh words -> plain write, armed
    words -> accumulate).
  - stage 2 (W-conv) per row window w: 16 matmuls of [128, 32]
    stationaries (from the ACT-copied SBUF mirror t) at psum partition
    offsets 32*hb into a per-window bank pre-zeroed two batches ahead
    (cheap matmul streaming the band's all-zero region), then one
    abs+sum reduction into the accumulator.  PSUM: 3 rotating T tags +
    5 rotating ps2 tags = 8 banks, with every rotation >= 1 batch clear
    of its previous tenant.
  - batch 7 runs in two half-strip chains (g-pairs) and drains windows
    5..7 with split DVE/ACT reductions; accumulator columns ship per
    image on the idle HWDGE rings.
  - Charbonnier: sqrt(d^2 + 1e-6) == |d| to ~1e-5 relative here.
  - The band is bf16(1/7) per stage; the host divides it back out and
    applies the exact 1/49.  The column bookkeeping is exact: stage-1
    stationaries are contiguous column blocks, so stage-2's contraction
    pairs column 128g+p with band(128g+p, n): true per-pixel conv.
"""

import numpy as np

import concourse.bass as bass
import concourse.bacc as bacc
import concourse.mybir as mybir
import concourse.tile as tile
from concourse.bass_interp import get_hw_module
from concourse.bass_utils import run_bass_kernel_spmd

N_CORES = 8
B_TOTAL = 16
B_PER_CORE = B_TOTAL // N_CORES
CH = 3
H = W = 512
P = 128
NC4 = 4
NSTRIP = B_PER_CORE * NC4  # 8 global strips / windows
EPS = 1e-6
F32 = mybir.dt.float32
BF16 = mybir.dt.bfloat16
# bf16 rounding of 1/7 (one factor per conv stage); host divides it back out
BAND_BF16 = 0.142578125
AF = mybir.ActivationFunctionType
BANDW = 520  # band free width: live window [0,136) + zeros through 520


def build_program():
    nc = bacc.Bacc("TRN2", target_bir_lowering=False, debug=False, num_devices=N_CORES)

    x = nc.dram_tensor("x", [B_PER_CORE, CH, H, W], F32, kind="ExternalInput")
    y = nc.dram_tensor("y", [B_PER_CORE, CH, H, W], F32, kind="ExternalInput")
    OUT_COLS = B_PER_CORE * 6
    out = nc.dram_tensor("out", [P, OUT_COLS], F32, kind="ExternalOutput")

    with tile.TileContext(nc) as tc:
        with (
            tc.tile_pool(name="const", bufs=1) as cpool,
            tc.tile_pool(name="xy", bufs=1) as xypool,
            tc.tile_pool(name="data", bufs=2) as dpool,
            tc.tile_pool(name="small", bufs=2) as spool,
            tc.tile_pool(name="psum", bufs=1, space="PSUM") as ppool,
        ):
            # ---- input DMAs: SWDGE cast-DMAs to bf16 ----
            xt, yt = [], []
            for b in range(B_PER_CORE):
                xb = xypool.tile([P, CH, NC4, W], BF16, name=f"xb{b}", tag=f"x{b}")
                yb = xypool.tile([P, CH, NC4, W], BF16, name=f"yb{b}", tag=f"y{b}")
                xt.append(xb)
                yt.append(yb)
            def issue_strip(b, c):
                src_x = x.ap()[b].rearrange("ch (c p) w -> p ch c w", c=NC4)
                src_y = y.ap()[b].rearrange("ch (c p) w -> p ch c w", c=NC4)
                if b == B_PER_CORE - 1 and c == NC4 - 1:
                    # last strip: one half + two quarters, so the final
                    # g2/g3 chains overlap the last transfers
                    for w0, w1 in ((0, 256), (256, 384), (384, 512)):
                        cs = slice(w0, w1)
                        nc.gpsimd.dma_start(
                            xt[b][:, :, c, cs], src_x[:, :, c, cs])
                        nc.gpsimd.dma_start(
                            yt[b][:, :, c, cs], src_y[:, :, c, cs])
                else:
                    nc.gpsimd.dma_start(xt[b][:, :, c, :], src_x[:, :, c, :])
                    nc.gpsimd.dma_start(yt[b][:, :, c, :], src_y[:, :, c, :])

            issue_strip(0, 0)
            issue_strip(0, 1)
            # ---- band: after 2 strip-pairs' issues, before the rest ----
            sev = cpool.tile([P, 1], F32, name="sev")
            nc.gpsimd.memset(sev[:], BAND_BF16)
            band = cpool.tile([P, BANDW], BF16, name="band")
            btmp = cpool.tile([P, BANDW], BF16, name="btmp")
            ge = mybir.AluOpType.is_ge
            # keep where p - j + 7 >= 0
            nc.gpsimd.affine_select(
                btmp[:], sev[:].to_broadcast([P, BANDW]),
                pattern=[[-1, BANDW]], base=7, channel_multiplier=1,
                compare_op=ge, fill=0.0,
            )
            # keep where -p + j - 1 >= 0
            nc.gpsimd.affine_select(
                band[:], btmp[:],
                pattern=[[1, BANDW]], base=-1, channel_multiplier=-1,
                compare_op=ge, fill=0.0,
            )

            for b in range(B_PER_CORE):
                for c in range(NC4):
                    if not (b == 0 and c < 2):
                        issue_strip(b, c)

            acc = cpool.tile([P, 12], F32, name="acc")
            col = 0
            out_col = 0

            prev = {}

            def ordered(key, inst):
                # pin each engine's queue to data-arrival order
                if key in prev:
                    tile.add_dep_helper(inst.ins, prev[key], sync=False,
                                        reason=f"{key} arrival order")
                prev[key] = inst.ins
                return inst

            # per-image s (diff) and t (H-conv mirror) tiles
            st = [dpool.tile([P, NC4, W], BF16, name=f"s{b}", tag="s")
                  for b in range(B_PER_CORE)]
            tt = [dpool.tile([P, NC4, W], BF16, name=f"t{b}", tag="t")
                  for b in range(B_PER_CORE)]
            Tg = [None] * NSTRIP   # stage-1 psum bank per global strip
            ps2 = [None] * NSTRIP  # stage-2 psum bank per global window

            def diff_strip(k, w0, w1):
                b, c = divmod(k, NC4)
                xb, yb, sv = xt[b], yt[b], st[b]
                ww = w1 - w0
                d0 = spool.tile([P, ww], BF16, name="d0", tag=f"d0_{ww}")
                d1 = spool.tile([P, ww], BF16, name="d1", tag=f"d1_{ww}")
                e = spool.tile([P, ww], BF16, name="e", tag=f"e_{ww}")
                ordered("v", nc.vector.tensor_sub(
                    d0[:], xb[:, 0, c, w0:w1], yb[:, 0, c, w0:w1]))
                ordered("v", nc.vector.tensor_sub(
                    d1[:], xb[:, 1, c, w0:w1], yb[:, 1, c, w0:w1]))
                ordered("v", nc.vector.tensor_add(e[:], d0[:], d1[:]))
                ordered("v", nc.vector.tensor_sub(
                    d1[:], xb[:, 2, c, w0:w1], yb[:, 2, c, w0:w1]))
                ordered("v", nc.vector.tensor_add(sv[:, c, w0:w1], e[:], d1[:]))

            def zero_ps2(w):
                ps2[w] = ppool.tile([P, W], F32, name=f"ps2_{w}",
                                    tag=f"ps2{w % 5}")
                # band[:, 136:264] is all zeros
                ordered("t", nc.tensor.matmul(
                    ps2[w][:], band[:, 136:264], band[:, 0:W],
                    start=True, stop=False,
                ))

            def stage1(k, g_lo, g_hi):
                b, c = divmod(k, NC4)
                s = st[b]
                if g_lo == 0:
                    Tg[k] = ppool.tile([P, NC4, P], F32, name=f"T{k}",
                                       tag=f"T{k % 3}")
                for g in range(g_lo, g_hi):
                    ordered("t", nc.tensor.matmul(
                        Tg[k][:, g, :],
                        s[:, c, 128 * g:128 * (g + 1)],
                        band[:, 4:132],
                        start=(g == 0),
                        stop=False,
                    ))
                if c > 0:
                    for g in range(g_lo, g_hi):
                        # strip c-1 rows 125..127 -> our rows 0..2
                        ordered("t", nc.tensor.matmul(
                            Tg[k][:, g, 0:3],
                            s[:, c - 1, 128 * g:128 * (g + 1)],
                            band[:, 132:135],
                            start=False,
                            stop=(c == NC4 - 1),
                        ))
                    for g in range(g_lo, g_hi):
                        # our rows 0..2 -> strip c-1 rows 125..127
                        ordered("t", nc.tensor.matmul(
                            Tg[k - 1][:, g, 125:128],
                            s[:, c, 128 * g:128 * (g + 1)],
                            band[:, 1:4],
                            start=False,
                            stop=True,
                        ))

            def copies(w, g_lo, g_hi, split=False):
                # T_w (final) -> t rows [128c, 128c+128); ACT mid-stream,
                # split DVE/ACT in the tail where the DVE is idle
                b, c = divmod(w, NC4)
                for g in range(g_lo, g_hi):
                    dst = tt[b][:, g, 128 * c:128 * (c + 1)]
                    if split and g % 2 == 0:
                        ordered("v", nc.vector.tensor_copy(dst, Tg[w][:, g, :]))
                    else:
                        ordered("s", nc.scalar.copy(dst, Tg[w][:, g, :]))

            def stage2(w, g_lo, g_hi):
                b, c = divmod(w, NC4)
                t = tt[b]
                for g in range(g_lo, g_hi):
                    n0, n1 = max(0, 128 * g - 4), min(W, 128 * g + 132)
                    j0 = n0 - 128 * g + 4
                    j1 = n1 - 128 * g + 4
                    for hb in range(NC4):
                        ordered("t", nc.tensor.matmul(
                            ps2[w][32 * hb:32 * hb + 32, n0:n1],
                            t[:, g, 128 * c + hb:128 * (c + 1):NC4],
                            band[:, j0:j1],
                            start=False,
                            stop=(hb == NC4 - 1 and g == NC4 - 1),
                            tile_position=(0, 32 * hb),
                        ))

            def reduce_window(w, split):
                # mid-stream reductions live on ACT so the DVE runs ONLY
                # diffs and paces the arrivals; tail windows split DVE/ACT.
                # Both engines write disjoint columns of ONE acc tile so a
                # single sync-ring DMA ships everything.
                nonlocal col
                if split:
                    ordered("v", nc.vector.tensor_reduce(
                        acc[:, col:col + 1], ps2[w][:, 0:W // 2],
                        axis=mybir.AxisListType.X, op=mybir.AluOpType.add,
                        apply_absolute_value=True))
                    col += 1
                    u = spool.tile([P, W // 2], BF16, name="u", tag="u")
                    ordered("s", nc.scalar.activation(
                        u[:], ps2[w][:, W // 2:], AF.Abs,
                        accum_out=acc[:, col:col + 1]))
                    col += 1
                else:
                    u = spool.tile([P, W], BF16, name="u2", tag="u2")
                    ordered("s", nc.scalar.activation(
                        u[:], ps2[w][:], AF.Abs,
                        accum_out=acc[:, col:col + 1]))
                    col += 1

            def ship_image(b, c_start):
                nonlocal out_col
                n = col - c_start
                if n:
                    nc.sync.dma_start(
                        out.ap()[:, out_col:out_col + n],
                        acc[:, c_start:col])
                    out_col += n

            img_marks = [0]
            # ---- steady-state batches k = 0..6 ----
            # PE emission order puts stage2 w(k-2) BEFORE stage1 k: its
            # inputs (copies from batch k-1) are already in SBUF, so the
            # PE works through it while the DVE is still diffing strip k.
            for k in range(NSTRIP - 1):
                if k + 2 < NSTRIP:
                    zero_ps2(k + 2)
                if k == 0:
                    zero_ps2(0)
                    zero_ps2(1)
                diff_strip(k, 0, W)
                if k >= 2:
                    stage2(k - 2, 0, NC4)
                stage1(k, 0, NC4)
                if k >= 1:
                    copies(k - 1, 0, NC4)
                if k >= 2:
                    reduce_window(k - 2, split=False)
                if k == NC4 + 1:
                    # img0's last window (w3) just reduced: ship img0
                    img_marks.append(col)
                    ship_image(0, 0)

            # ---- tail batch k = 7: two half-strip chains ----
            # DVE runs both half-diffs back to back (second waits only on
            # its own DMA), then the drain copies/reductions; the PE
            # dovetails stage2 w5 into the gap between the half stage-1s.
            kL = NSTRIP - 1
            diff_strip(kL, 0, W // 2)            # DVE: half a (g01)
            diff_strip(kL, W // 2, 3 * W // 4)   # DVE: quarter c (g2)
            stage1(kL, 0, 2)                     # PE (after diff a)
            stage2(kL - 2, 0, NC4)               # PE: w5 (copies from batch 6)
            copies(kL - 1, 0, 2, split=True)     # w6 g01
            copies(kL, 0, 2, split=True)         # w7 g01
            diff_strip(kL, 3 * W // 4, W)        # DVE: quarter d (g3)
            stage1(kL, 2, 3)                     # PE (after diff c)
            stage2(kL - 1, 0, 2)                 # w6 g01
            stage2(kL, 0, 2)                     # w7 g01
            copies(kL - 1, 2, 3, split=True)     # w6 g2 (DVE)
            copies(kL, 2, 3, split=True)         # w7 g2 (DVE)
            stage1(kL, 3, NC4)                   # PE (after diff d)
            stage2(kL - 1, 2, 3)
            stage2(kL, 2, 3)
            copies(kL - 1, 3, NC4, split=True)   # w6 g3 (ACT)
            copies(kL, 3, NC4, split=True)       # w7 g3 (ACT)
            stage2(kL - 1, 3, NC4)
            stage2(kL, 3, NC4)
            reduce_window(kL - 2, split=True)    # w5
            reduce_window(kL - 1, split=True)    # w6
            reduce_window(kL, split=True)        # w7
            ship_image(1, img_marks[1])

            n_out_cols = out_col

    nc.compile()
    nc.m = get_hw_module(nc.m)
    return nc, x.name, y.name, out.name, n_out_cols


_CACHE = {}


def _get_program():
    if "prog" not in _CACHE:
        _CACHE["prog"] = build_program()
    return _CACHE["prog"]


def run_sharded(x: np.ndarray, y: np.ndarray, trace: bool = False):
    """Run the SPMD kernel; returns (per-core sums list, BassKernelResults)."""
    nc, xname, yname, outname, n_cols = _get_program()
    x = np.ascontiguousarray(np.asarray(x, dtype=np.float32))
    y = np.ascontiguousarray(np.asarray(y, dtype=np.float32))
    in_maps = []
    for k in range(N_CORES):
        sl = slice(k * B_PER_CORE, (k + 1) * B_PER_CORE)
        in_maps.append({
            xname: x[sl],
            yname: y[sl],
        })
    res = run_bass_kernel_spmd(
        nc, in_maps, core_ids=list(range(N_CORES)), trace=trace
    )
    sums = [float(res.results[k][outname][:, :n_cols]
                  .astype(np.float64).sum())
            for k in range(N_CORES)]
    return sums, res


def kernel(x: np.ndarray, y: np.ndarray) -> np.ndarray:
    sums, _ = run_sharded(x, y)
    total = float(np.sum(np.asarray(sums, dtype=np.float64)))
    # the device band carries bf16(1/7) per conv stage; divide it back out
    # and apply the exact 1/49 here
    total *= (1.0 / 49.0) / (BAND_BF16 * BAND_BF16)
    return np.float32(total / (B_TOTAL * H * W))
